# revision 11
# baseline (speedup 1.0000x reference)
"""Trainium2 Bass kernel for a 2-layer minGRU (B=8, S=4096, D=H=512).

Sharding: data-parallel over batch B across 8 NeuronCores (core b gets x[b]);
weights replicated.

Per-core layout: channels on SBUF partitions, sequence on the free dim.
 - gate preactivations k = W @ x^T computed with PE matmuls (contract over
   input channels, 4 K-chunks of 128), accumulated into [128, 1024] PSUM
   tiles (two 512-col bank halves per tile).
 - a  = sigmoid(-(k+bz)) = 1-z          (ACT, bias/scale fused, from PSUM)
 - s  = sigmoid(p+bh)                   (ACT)
 - h~ = max(p + (bh+0.5), s)            (DVE scalar_tensor_tensor; exact
                                         identity for the reference g())
 - z  = 1-a                             (GPSIMD tensor_scalar)
 - b  = z*h~                            (GPSIMD tensor_tensor)
 - h_t = a_t*h_{t-1} + b_t via DVE tensor_tensor_scan along the free dim,
   chained across 1024-wide blocks via initial=prev[:, -1:].
Layer 2 consumes layer-1's hidden states directly ([H,S] layout is already
the moving-operand layout for the next matmul) and is pipelined behind
layer 1 at block granularity.
"""

import numpy as np
import ml_dtypes

import concourse.bass as bass
import concourse.tile as tile
from concourse import bacc, mybir
from concourse.bass_utils import run_bass_kernel_spmd

B, S, D, H, L = 8, 4096, 512, 512, 2
P = 128
HC = H // P        # 4 output-channel chunks
KC = D // P        # 4 contraction chunks
T = 512            # matmul moving free dim (one PSUM bank)
BW = 1024          # block width for PSUM tiles / ACT / scan
NB = S // BW       # 4 blocks
HPB = BW // T      # 2 matmul column-halves per block

F32 = mybir.dt.float32
BF16 = mybir.dt.bfloat16
AF = mybir.ActivationFunctionType
OP = mybir.AluOpType

N_CORES = 8

GPSIMD_Z = True   # compute z = 1-a on GPSIMD
GPSIMD_B = True   # compute b = z*h~ on GPSIMD


def _build():
    nc = bacc.Bacc("TRN2", target_bir_lowering=False, debug=False)

    # ---- DRAM I/O (per core) ----
    xT = nc.dram_tensor("xT", [D, S], BF16, kind="ExternalInput").ap()
    wT = {}
    for l in range(L):
        for g in ("z", "h"):
            wT[(l, g)] = nc.dram_tensor(
                f"w{g}T{l}", [H, H], BF16, kind="ExternalInput"
            ).ap()
    bias_d = {}
    for l in range(L):
        for nm in ("nbz", "bh", "bp"):
            bias_d[(l, nm)] = nc.dram_tensor(
                f"{nm}{l}", [P, HC], F32, kind="ExternalInput"
            ).ap()
    h2T = nc.dram_tensor("h2T", [H, S], BF16, kind="ExternalOutput").ap()
    fin = nc.dram_tensor("fin", [L, H], BF16, kind="ExternalOutput").ap()

    with tile.TileContext(nc) as tc:
        with (
            tc.tile_pool(name="const", bufs=1) as cpool,
            tc.tile_pool(name="xp", bufs=4) as xpool,
            tc.tile_pool(name="h1p", bufs=3) as h1pool,
            tc.tile_pool(name="h2p", bufs=2) as h2pool,
            tc.tile_pool(name="ab", bufs=2) as abpool,
            tc.tile_pool(name="wk", bufs=3) as wk,
            tc.tile_pool(name="ps", bufs=2, space="PSUM") as pp,
        ):
            # ---- weights + biases to SBUF (layer-0 weights first) ----
            w_sb = {}
            bias_sb = {}

            def load_layer_consts(l):
                for g in ("z", "h"):
                    t = cpool.tile([P, KC, H], BF16, tag=f"w{g}{l}")
                    nc.sync.dma_start(
                        t[:], wT[(l, g)].rearrange("(c p) h -> p c h", p=P)
                    )
                    w_sb[(l, g)] = t
                for nm in ("nbz", "bh", "bp"):
                    t = cpool.tile([P, HC], F32, tag=f"{nm}{l}")
                    nc.sync.dma_start(t[:], bias_d[(l, nm)])
                    bias_sb[(l, nm)] = t

            load_layer_consts(0)

            # ---- x tiles (per K-chunk, per 512-col tile) ----
            xt = {}

            def load_x(sj):
                for d in range(KC):
                    t = xpool.tile([P, T], BF16, tag=f"x{d}")
                    nc.sync.dma_start(
                        t[:], xT[d * P : (d + 1) * P, sj * T : (sj + 1) * T]
                    )
                    xt[(d, sj)] = t

            load_x(0)
            load_x(1)
            load_layer_consts(1)
            for sj in range(2, S // T):
                load_x(sj)

            h1_tiles = {}          # (hc, blk) -> [P, BW] bf16
            h2_prev = [None] * HC

            def emit_layer(l, blk):
                for hc in range(HC):
                    kz = pp.tile([P, BW], F32, tag="psA")
                    kh = pp.tile([P, BW], F32, tag="psB")
                    for g, acc in (("z", kz), ("h", kh)):
                        for half in range(HPB):
                            sj = blk * HPB + half
                            for d in range(KC):
                                lhsT = w_sb[(l, g)][:, d, hc * P : (hc + 1) * P]
                                if l == 0:
                                    rhs = xt[(d, sj)][:]
                                else:
                                    rhs = h1_tiles[(d, blk)][:, half * T : (half + 1) * T]
                                nc.tensor.matmul(
                                    acc[:, half * T : (half + 1) * T], lhsT, rhs,
                                    start=(d == 0), stop=(d == KC - 1),
                                )
                    nbz = bias_sb[(l, "nbz")][:, hc : hc + 1]
                    bh = bias_sb[(l, "bh")][:, hc : hc + 1]
                    bp = bias_sb[(l, "bp")][:, hc : hc + 1]

                    a = abpool.tile([P, BW], BF16, tag=f"a{hc}")
                    nc.scalar.activation(a[:], kz[:], AF.Sigmoid, bias=nbz, scale=-1.0)
                    s = wk.tile([P, BW], BF16, tag="s")
                    nc.scalar.activation(s[:], kh[:], AF.Sigmoid, bias=bh)
                    ht = wk.tile([P, BW], BF16, tag="ht")
                    # ht = max(kh + (bh+0.5), s)  == g(kh+bh) given s=sigmoid(kh+bh)
                    nc.vector.scalar_tensor_tensor(
                        ht[:], kh[:], bp, s[:], op0=OP.add, op1=OP.max
                    )
                    z = wk.tile([P, BW], BF16, tag="z")
                    zeng = nc.gpsimd if GPSIMD_Z else nc.vector
                    zeng.tensor_scalar(z[:], a[:], -1.0, 1.0, OP.mult, OP.add)
                    b = wk.tile([P, BW], BF16, tag="b")
                    beng = nc.gpsimd if GPSIMD_B else nc.vector
                    beng.tensor_tensor(b[:], z[:], ht[:], op=OP.mult)

                    if l == 0:
                        out_t = h1pool.tile([P, BW], BF16, tag=f"h1_{hc}")
                        init = 0.5 if blk == 0 else h1_tiles[(hc, blk - 1)][:, BW - 1 : BW]
                        nc.vector.tensor_tensor_scan(
                            out_t[:], a[:], b[:], init, op0=OP.mult, op1=OP.add
                        )
                        h1_tiles[(hc, blk)] = out_t
                    else:
                        out_t = h2pool.tile([P, BW], BF16, tag=f"h2_{hc}")
                        init = 0.5 if blk == 0 else h2_prev[hc][:, BW - 1 : BW]
                        nc.vector.tensor_tensor_scan(
                            out_t[:], a[:], b[:], init, op0=OP.mult, op1=OP.add
                        )
                        h2_prev[hc] = out_t
                        nc.sync.dma_start(
                            h2T[hc * P : (hc + 1) * P, blk * BW : (blk + 1) * BW],
                            out_t[:],
                        )
                        if blk == NB - 1:
                            nc.sync.dma_start(
                                fin[1, hc * P : (hc + 1) * P],
                                out_t[:, BW - 1 : BW],
                            )

            # pipeline: L1(blk) then L2(blk-1), so PE always has L1 work queued
            # ahead of the L2 group that waits on L1(blk)'s scan.
            for blk in range(NB):
                emit_layer(0, blk)
                if blk >= 1:
                    emit_layer(1, blk - 1)
            for hc in range(HC):
                nc.sync.dma_start(
                    fin[0, hc * P : (hc + 1) * P],
                    h1_tiles[(hc, NB - 1)][:, BW - 1 : BW],
                )
            emit_layer(1, NB - 1)

    nc.compile()
    return nc


_nc_cache = None
LAST_RESULTS = None  # BassKernelResults of the most recent run (for test.py)


def _get_nc():
    global _nc_cache
    if _nc_cache is None:
        _nc_cache = _build()
    return _nc_cache


def _chunk_bias(v):
    # (512,) -> [128, 4] where column c is channels [c*128, (c+1)*128)
    return np.ascontiguousarray(v.reshape(HC, P).T.astype(np.float32))


def kernel(x, wz0, bz0, wh0, bh0, wz1, bz1, wh1, bh1, **_):
    x = np.asarray(x, dtype=np.float32)
    nc = _get_nc()

    bf = ml_dtypes.bfloat16
    common = {
        "wzT0": np.ascontiguousarray(np.asarray(wz0).T.astype(bf)),
        "whT0": np.ascontiguousarray(np.asarray(wh0).T.astype(bf)),
        "wzT1": np.ascontiguousarray(np.asarray(wz1).T.astype(bf)),
        "whT1": np.ascontiguousarray(np.asarray(wh1).T.astype(bf)),
        "nbz0": _chunk_bias(-np.asarray(bz0)),
        "bh0": _chunk_bias(np.asarray(bh0)),
        "bp0": _chunk_bias(np.asarray(bh0) + 0.5),
        "nbz1": _chunk_bias(-np.asarray(bz1)),
        "bh1": _chunk_bias(np.asarray(bh1)),
        "bp1": _chunk_bias(np.asarray(bh1) + 0.5),
    }
    in_maps = [
        {**common, "xT": np.ascontiguousarray(x[b].T.astype(bf))} for b in range(B)
    ]
    res = run_bass_kernel_spmd(nc, in_maps, core_ids=list(range(N_CORES)))
    global LAST_RESULTS
    LAST_RESULTS = res

    out = np.empty((B, S, H), np.float32)
    fin = np.empty((L, B, 1, H), np.float32)
    for b in range(B):
        out[b] = res.results[b]["h2T"].astype(np.float32).T
        f = res.results[b]["fin"].astype(np.float32)
        fin[0, b, 0] = f[0]
        fin[1, b, 0] = f[1]
    return out, fin


# revision 13
# speedup vs baseline: 1.4226x; 1.4226x over previous
"""Trainium2 Bass kernel for a 2-layer minGRU (B=8, S=4096, D=H=512).

Sharding: data-parallel over batch B across 8 NeuronCores (core b gets x[b]);
weights replicated.

Per-core layout: channels on SBUF partitions, sequence on the free dim.
 - gate preactivations k = W @ x^T computed with PE matmuls (contract over
   input channels, 4 K-chunks of 128), accumulated into [128, 1024] PSUM
   tiles (two 512-col bank halves per tile).
 - a  = sigmoid(-(k+bz)) = 1-z            (ACT, bias/scale fused, from PSUM)
 - s  = sigmoid(p+bh)                     (ACT)
 - u  = relu(p+bh+0.5)                    (ACT)
 - h~ = max(s, u)                         (DVE TT; exact identity for g())
 - z  = 1-a                               (DVE tensor_scalar, 4x bf16)
 - b  = z*h~                              (DVE TT, 2x bf16)
 - h_t = a_t*h_{t-1} + b_t via DVE tensor_tensor_scan along the free dim
   (2048-wide scans; the final block of layer 2 is split 1024+1024 to
   shorten the post-matmul tail), chained via initial=prev[:, -1:].
Layer 2 consumes layer-1's hidden states directly ([H,S] layout is already
the moving-operand layout for the next matmul) and is pipelined behind
layer 1 at block granularity.
"""

import numpy as np
import ml_dtypes

import concourse.bass as bass
import concourse.tile as tile
from concourse import bacc, mybir
from concourse.bass_utils import run_bass_kernel_spmd

B, S, D, H, L = 8, 4096, 512, 512, 2
P = 128
HC = H // P        # 4 output-channel chunks
KC = D // P        # 4 contraction chunks
T = 512            # matmul moving free dim (one PSUM bank)
PW = 1024          # PSUM tile width (2 banks; ACT op width)

F32 = mybir.dt.float32
BF16 = mybir.dt.bfloat16
AF = mybir.ActivationFunctionType
OP = mybir.AluOpType

N_CORES = 8

# scan-block widths per layer (sum = S); last L2 blocks kept small to
# shorten the tail after the final matmuls
BLOCKS = {0: [2048, 2048], 1: [2048, 1024, 1024]}


def _build():
    nc = bacc.Bacc("TRN2", target_bir_lowering=False, debug=False)

    # ---- DRAM I/O (per core) ----
    xT = nc.dram_tensor("xT", [D, S], BF16, kind="ExternalInput").ap()
    wT = {}
    for l in range(L):
        for g in ("z", "h"):
            wT[(l, g)] = nc.dram_tensor(
                f"w{g}T{l}", [H, H], BF16, kind="ExternalInput"
            ).ap()
    bias_d = {}
    for l in range(L):
        for nm in ("nbz", "bh", "bp"):
            bias_d[(l, nm)] = nc.dram_tensor(
                f"{nm}{l}", [P, HC], F32, kind="ExternalInput"
            ).ap()
    h2T = nc.dram_tensor("h2T", [H, S], BF16, kind="ExternalOutput").ap()
    fin = nc.dram_tensor("fin", [L, H], BF16, kind="ExternalOutput").ap()

    with tile.TileContext(nc) as tc:
        with (
            tc.tile_pool(name="const", bufs=1) as cpool,
            tc.tile_pool(name="xp", bufs=6) as xpool,
            tc.tile_pool(name="h1p", bufs=2) as h1pool,
            tc.tile_pool(name="h2p", bufs=2) as h2pool,
            tc.tile_pool(name="ab", bufs=2) as abpool,
            tc.tile_pool(name="wk", bufs=3) as wk,
            tc.tile_pool(name="ps", bufs=2, space="PSUM") as pp,
        ):
            # ---- weights + biases to SBUF (on the gpsimd DMA queue so x
            # loads on sync start immediately) ----
            w_sb = {}
            bias_sb = {}

            def load_layer_consts(l):
                for g in ("z", "h"):
                    t = cpool.tile([P, KC, H], BF16, tag=f"w{g}{l}")
                    nc.gpsimd.dma_start(
                        t[:], wT[(l, g)].rearrange("(c p) h -> p c h", p=P)
                    )
                    w_sb[(l, g)] = t
                for nm in ("nbz", "bh", "bp"):
                    t = cpool.tile([P, HC], F32, tag=f"{nm}{l}")
                    nc.gpsimd.dma_start(t[:], bias_d[(l, nm)])
                    bias_sb[(l, nm)] = t

            load_layer_consts(0)
            load_layer_consts(1)

            # ---- x tiles (per K-chunk, per 512-col tile) ----
            xt = {}
            for sj in range(S // T):
                for d in range(KC):
                    t = xpool.tile([P, T], BF16, tag=f"x{d}")
                    nc.sync.dma_start(
                        t[:], xT[d * P : (d + 1) * P, sj * T : (sj + 1) * T]
                    )
                    xt[(d, sj)] = t

            h1_tiles = {}          # (hc, blk_idx) -> ([P, bw] bf16, start_col)
            h2_prev = [None] * HC

            def emit_layer(l, bi):
                bw = BLOCKS[l][bi]
                s0 = sum(BLOCKS[l][:bi])
                for hc in range(HC):
                    nbz = bias_sb[(l, "nbz")][:, hc : hc + 1]
                    bh = bias_sb[(l, "bh")][:, hc : hc + 1]
                    bp = bias_sb[(l, "bp")][:, hc : hc + 1]

                    a2 = abpool.tile([P, 2048], BF16, tag=f"a{hc}")
                    ht2 = wk.tile([P, 2048], BF16, tag="ht")
                    for off in range(0, bw, PW):
                        pw = min(PW, bw - off)
                        kz = pp.tile([P, PW], F32, tag="psA")
                        kh = pp.tile([P, PW], F32, tag="psB")
                        for g, acc in (("z", kz), ("h", kh)):
                            for half in range(pw // T):
                                sj = (s0 + off) // T + half
                                for d in range(KC):
                                    lhsT = w_sb[(l, g)][:, d, hc * P : (hc + 1) * P]
                                    if l == 0:
                                        rhs = xt[(d, sj)][:]
                                    else:
                                        src, sc = h1_tiles[(d, 0 if sj < 4 else 1)]
                                        rhs = src[:, sj * T - sc : (sj + 1) * T - sc]
                                    nc.tensor.matmul(
                                        acc[:, half * T : (half + 1) * T], lhsT, rhs,
                                        start=(d == 0), stop=(d == KC - 1),
                                    )
                        nc.scalar.activation(
                            a2[:, off : off + pw], kz[:, :pw], AF.Sigmoid,
                            bias=nbz, scale=-1.0,
                        )
                        s = wk.tile([P, PW], BF16, tag="s")
                        nc.scalar.activation(s[:, :pw], kh[:, :pw], AF.Sigmoid, bias=bh)
                        u = wk.tile([P, PW], BF16, tag="u")
                        nc.scalar.activation(u[:, :pw], kh[:, :pw], AF.Relu, bias=bp)
                        nc.vector.tensor_tensor(
                            ht2[:, off : off + pw], s[:, :pw], u[:, :pw], op=OP.max
                        )
                    z2 = wk.tile([P, 2048], BF16, tag="z")
                    nc.vector.tensor_scalar(
                        z2[:, :bw], a2[:, :bw], -1.0, 1.0, OP.mult, OP.add
                    )
                    b2 = wk.tile([P, 2048], BF16, tag="b")
                    nc.vector.tensor_mul(b2[:, :bw], z2[:, :bw], ht2[:, :bw])

                    if l == 0:
                        out_t = h1pool.tile([P, 2048], BF16, tag=f"h1_{hc}")
                        init = 0.5 if bi == 0 else h1_tiles[(hc, bi - 1)][0][:, 2047:2048]
                        nc.vector.tensor_tensor_scan(
                            out_t[:, :bw], a2[:, :bw], b2[:, :bw], init,
                            op0=OP.mult, op1=OP.add,
                        )
                        h1_tiles[(hc, bi)] = (out_t, s0)
                    else:
                        out_t = h2pool.tile([P, 2048], BF16, tag=f"h2_{hc}")
                        if bi == 0:
                            init = 0.5
                        else:
                            pt, pbw = h2_prev[hc]
                            init = pt[:, pbw - 1 : pbw]
                        nc.vector.tensor_tensor_scan(
                            out_t[:, :bw], a2[:, :bw], b2[:, :bw], init,
                            op0=OP.mult, op1=OP.add,
                        )
                        h2_prev[hc] = (out_t, bw)
                        nc.sync.dma_start(
                            h2T[hc * P : (hc + 1) * P, s0 : s0 + bw],
                            out_t[:, :bw],
                        )
                        if bi == len(BLOCKS[1]) - 1:
                            nc.sync.dma_start(
                                fin[1, hc * P : (hc + 1) * P],
                                out_t[:, bw - 1 : bw],
                            )

            # pipeline: L1(bi) then L2(bi-1): PE always has L1 work queued
            # ahead of the L2 group that waits on L1(bi)'s scan.
            emit_layer(0, 0)
            emit_layer(0, 1)
            emit_layer(1, 0)
            for hc in range(HC):
                nc.sync.dma_start(
                    fin[0, hc * P : (hc + 1) * P],
                    h1_tiles[(hc, 1)][0][:, 2047:2048],
                )
            emit_layer(1, 1)
            emit_layer(1, 2)

    nc.compile()
    return nc


_nc_cache = None
LAST_RESULTS = None  # BassKernelResults of the most recent run (for test.py)


def _get_nc():
    global _nc_cache
    if _nc_cache is None:
        _nc_cache = _build()
    return _nc_cache


def _chunk_bias(v):
    # (512,) -> [128, 4] where column c is channels [c*128, (c+1)*128)
    return np.ascontiguousarray(v.reshape(HC, P).T.astype(np.float32))


def kernel(x, wz0, bz0, wh0, bh0, wz1, bz1, wh1, bh1, **_):
    x = np.asarray(x, dtype=np.float32)
    nc = _get_nc()

    bf = ml_dtypes.bfloat16
    common = {
        "wzT0": np.ascontiguousarray(np.asarray(wz0).T.astype(bf)),
        "whT0": np.ascontiguousarray(np.asarray(wh0).T.astype(bf)),
        "wzT1": np.ascontiguousarray(np.asarray(wz1).T.astype(bf)),
        "whT1": np.ascontiguousarray(np.asarray(wh1).T.astype(bf)),
        "nbz0": _chunk_bias(-np.asarray(bz0)),
        "bh0": _chunk_bias(np.asarray(bh0)),
        "bp0": _chunk_bias(np.asarray(bh0) + 0.5),
        "nbz1": _chunk_bias(-np.asarray(bz1)),
        "bh1": _chunk_bias(np.asarray(bh1)),
        "bp1": _chunk_bias(np.asarray(bh1) + 0.5),
    }
    in_maps = [
        {**common, "xT": np.ascontiguousarray(x[b].T.astype(bf))} for b in range(B)
    ]
    res = run_bass_kernel_spmd(nc, in_maps, core_ids=list(range(N_CORES)))
    global LAST_RESULTS
    LAST_RESULTS = res

    out = np.empty((B, S, H), np.float32)
    fin = np.empty((L, B, 1, H), np.float32)
    for b in range(B):
        out[b] = res.results[b]["h2T"].astype(np.float32).T
        f = res.results[b]["fin"].astype(np.float32)
        fin[0, b, 0] = f[0]
        fin[1, b, 0] = f[1]
    return out, fin


# revision 18
# speedup vs baseline: 1.6161x; 1.1361x over previous
"""Trainium2 Bass kernel for a 2-layer minGRU (B=8, S=4096, D=H=512).

Sharding: data-parallel over batch B across 8 NeuronCores (core b gets x[b]);
weights replicated.

Per-core layout: channels on SBUF partitions, sequence on the free dim.
 - gate preactivations k = W @ x^T computed with PE matmuls (contract over
   input channels, 4 K-chunks of 128), accumulated into [128, 1024] PSUM
   tiles (two 512-col bank halves per tile).
 - a  = sigmoid(-(k+bz)) = 1-z            (ACT, bias/scale fused, from PSUM)
 - s  = sigmoid(p+bh)                     (ACT)
 - u  = relu(p+bh+0.5)                    (ACT)
 - h~ = max(s, u)                         (DVE TT; exact identity for g())
 - z  = 1-a                               (DVE tensor_scalar, 4x bf16)
 - b  = z*h~                              (DVE TT, 2x bf16)
 - h_t = a_t*h_{t-1} + b_t via DVE tensor_tensor_scan along the free dim
   (2048-wide scans; the final block of layer 2 is split 1024+1024 to
   shorten the post-matmul tail), chained via initial=prev[:, -1:].
Layer 2 consumes layer-1's hidden states directly ([H,S] layout is already
the moving-operand layout for the next matmul) and is pipelined behind
layer 1 at block granularity.
"""

import numpy as np
import ml_dtypes

import concourse.bass as bass
import concourse.tile as tile
from concourse import bacc, mybir
from concourse.bass_utils import run_bass_kernel_spmd

B, S, D, H, L = 8, 4096, 512, 512, 2
P = 128
HC = H // P        # 4 output-channel chunks
KC = D // P        # 4 contraction chunks
T = 512            # matmul moving free dim (one PSUM bank)
PW = 1024          # PSUM tile width (2 banks; ACT op width)

F32 = mybir.dt.float32
BF16 = mybir.dt.bfloat16
AF = mybir.ActivationFunctionType
OP = mybir.AluOpType

N_CORES = 8

# scan-block widths per layer (sum = S); last L2 blocks kept small to
# shorten the tail after the final matmuls
BLOCKS = {0: [2048, 2048], 1: [2048, 1024, 512, 512]}
TAILW = 512  # trailing h1 columns exported for the layer-1 finals


def _build():
    nc = bacc.Bacc("TRN2", target_bir_lowering=False, debug=False)

    # ---- DRAM I/O (per core) ----
    xT = nc.dram_tensor("xT", [D, S], BF16, kind="ExternalInput").ap()
    wT = {}
    for l in range(L):
        for g in ("z", "h"):
            wT[(l, g)] = nc.dram_tensor(
                f"w{g}T{l}", [H, H], BF16, kind="ExternalInput"
            ).ap()
    bias_d = {}
    for l in range(L):
        for nm in ("nbz", "bh", "bp"):
            bias_d[(l, nm)] = nc.dram_tensor(
                f"{nm}{l}", [P, HC], F32, kind="ExternalInput"
            ).ap()
    h2T = nc.dram_tensor("h2T", [H, S], BF16, kind="ExternalOutput").ap()
    h1tail = nc.dram_tensor("h1tail", [H, TAILW], BF16, kind="ExternalOutput").ap()

    with tile.TileContext(nc) as tc:
        with (
            tc.tile_pool(name="const", bufs=1) as cpool,
            tc.tile_pool(name="xp", bufs=6) as xpool,
            tc.tile_pool(name="h1p", bufs=2) as h1pool,
            tc.tile_pool(name="h2p", bufs=2) as h2pool,
            tc.tile_pool(name="ab", bufs=2) as abpool,
            tc.tile_pool(name="wk", bufs=3) as wk,
            tc.tile_pool(name="ps", bufs=2, space="PSUM") as pp,
        ):
            # ---- weights + biases to SBUF (on the gpsimd DMA queue so x
            # loads on sync start immediately) ----
            w_sb = {}
            bias_sb = {}

            def load_layer_consts(l):
                for g in ("z", "h"):
                    t = cpool.tile([P, KC, H], BF16, tag=f"w{g}{l}")
                    nc.gpsimd.dma_start(
                        t[:], wT[(l, g)].rearrange("(c p) h -> p c h", p=P)
                    )
                    w_sb[(l, g)] = t
                for nm in ("nbz", "bh", "bp"):
                    t = cpool.tile([P, HC], F32, tag=f"{nm}{l}")
                    nc.gpsimd.dma_start(t[:], bias_d[(l, nm)])
                    bias_sb[(l, nm)] = t

            load_layer_consts(0)
            load_layer_consts(1)

            # ---- x tiles (per K-chunk, per 512-col tile) ----
            xt = {}
            for sj in range(S // T):
                for d in range(KC):
                    t = xpool.tile([P, T], BF16, tag=f"x{d}")
                    nc.sync.dma_start(
                        t[:], xT[d * P : (d + 1) * P, sj * T : (sj + 1) * T]
                    )
                    xt[(d, sj)] = t

            h1_tiles = {}          # (hc, blk_idx) -> ([P, bw] bf16, start_col)
            h2_prev = [None] * HC

            def emit_layer(l, bi):
                bw = BLOCKS[l][bi]
                s0 = sum(BLOCKS[l][:bi])
                for hc in range(HC):
                    nbz = bias_sb[(l, "nbz")][:, hc : hc + 1]
                    bh = bias_sb[(l, "bh")][:, hc : hc + 1]
                    bp = bias_sb[(l, "bp")][:, hc : hc + 1]

                    a2 = abpool.tile([P, 2048], BF16, tag=f"a{hc}")
                    ht2 = wk.tile([P, 2048], BF16, tag="ht")
                    for off in range(0, bw, PW):
                        pw = min(PW, bw - off)
                        kz = pp.tile([P, PW], F32, tag="psA")
                        kh = pp.tile([P, PW], F32, tag="psB")
                        for g, acc in (("z", kz), ("h", kh)):
                            for half in range(pw // T):
                                sj = (s0 + off) // T + half
                                for d in range(KC):
                                    lhsT = w_sb[(l, g)][:, d, hc * P : (hc + 1) * P]
                                    if l == 0:
                                        rhs = xt[(d, sj)][:]
                                    else:
                                        src, sc = h1_tiles[(d, 0 if sj < 4 else 1)]
                                        rhs = src[:, sj * T - sc : (sj + 1) * T - sc]
                                    nc.tensor.matmul(
                                        acc[:, half * T : (half + 1) * T], lhsT, rhs,
                                        start=(d == 0), stop=(d == KC - 1),
                                    )
                        nc.scalar.activation(
                            a2[:, off : off + pw], kz[:, :pw], AF.Sigmoid,
                            bias=nbz, scale=-1.0,
                        )
                        s = wk.tile([P, PW], BF16, tag="s")
                        nc.scalar.activation(s[:, :pw], kh[:, :pw], AF.Sigmoid, bias=bh)
                        u = wk.tile([P, PW], BF16, tag="u")
                        nc.scalar.activation(u[:, :pw], kh[:, :pw], AF.Relu, bias=bp)
                        nc.vector.tensor_tensor(
                            ht2[:, off : off + pw], s[:, :pw], u[:, :pw], op=OP.max
                        )
                    z2 = wk.tile([P, 2048], BF16, tag="z")
                    nc.vector.tensor_scalar(
                        z2[:, :bw], a2[:, :bw], -1.0, 1.0, OP.mult, OP.add
                    )
                    b2 = wk.tile([P, 2048], BF16, tag="b")
                    nc.vector.tensor_mul(b2[:, :bw], z2[:, :bw], ht2[:, :bw])

                    if l == 0:
                        out_t = h1pool.tile([P, 2048], BF16, tag=f"h1_{hc}")
                        init = 0.5 if bi == 0 else h1_tiles[(hc, bi - 1)][0][:, 2047:2048]
                        nc.vector.tensor_tensor_scan(
                            out_t[:, :bw], a2[:, :bw], b2[:, :bw], init,
                            op0=OP.mult, op1=OP.add,
                        )
                        h1_tiles[(hc, bi)] = (out_t, s0)
                    else:
                        out_t = h2pool.tile([P, 2048], BF16, tag=f"h2_{hc}")
                        if bi == 0:
                            init = 0.5
                        else:
                            pt, pbw = h2_prev[hc]
                            init = pt[:, pbw - 1 : pbw]
                        nc.vector.tensor_tensor_scan(
                            out_t[:, :bw], a2[:, :bw], b2[:, :bw], init,
                            op0=OP.mult, op1=OP.add,
                        )
                        h2_prev[hc] = (out_t, bw)
                        nc.sync.dma_start(
                            h2T[hc * P : (hc + 1) * P, s0 : s0 + bw],
                            out_t[:, :bw],
                        )

            # pipeline: L1(bi) then L2(bi-1): PE always has L1 work queued
            # ahead of the L2 group that waits on L1(bi)'s scan.
            emit_layer(0, 0)
            emit_layer(0, 1)
            emit_layer(1, 0)
            for hc in range(HC):
                nc.sync.dma_start(
                    h1tail[hc * P : (hc + 1) * P, :],
                    h1_tiles[(hc, 1)][0][:, 2048 - TAILW : 2048],
                )
            for bi in range(1, len(BLOCKS[1])):
                emit_layer(1, bi)

    nc.compile()
    return nc


_nc_cache = None
LAST_RESULTS = None  # BassKernelResults of the most recent run (for test.py)


def _get_nc():
    global _nc_cache
    if _nc_cache is None:
        _nc_cache = _build()
    return _nc_cache


def _chunk_bias(v):
    # (512,) -> [128, 4] where column c is channels [c*128, (c+1)*128)
    return np.ascontiguousarray(v.reshape(HC, P).T.astype(np.float32))


def kernel(x, wz0, bz0, wh0, bh0, wz1, bz1, wh1, bh1, **_):
    x = np.asarray(x, dtype=np.float32)
    nc = _get_nc()

    bf = ml_dtypes.bfloat16
    common = {
        "wzT0": np.ascontiguousarray(np.asarray(wz0).T.astype(bf)),
        "whT0": np.ascontiguousarray(np.asarray(wh0).T.astype(bf)),
        "wzT1": np.ascontiguousarray(np.asarray(wz1).T.astype(bf)),
        "whT1": np.ascontiguousarray(np.asarray(wh1).T.astype(bf)),
        "nbz0": _chunk_bias(-np.asarray(bz0)),
        "bh0": _chunk_bias(np.asarray(bh0)),
        "bp0": _chunk_bias(np.asarray(bh0) + 0.5),
        "nbz1": _chunk_bias(-np.asarray(bz1)),
        "bh1": _chunk_bias(np.asarray(bh1)),
        "bp1": _chunk_bias(np.asarray(bh1) + 0.5),
    }
    in_maps = [
        {**common, "xT": np.ascontiguousarray(x[b].T.astype(bf))} for b in range(B)
    ]
    res = run_bass_kernel_spmd(nc, in_maps, core_ids=list(range(N_CORES)))
    global LAST_RESULTS
    LAST_RESULTS = res

    out = np.empty((B, S, H), np.float32)
    fin = np.empty((L, B, 1, H), np.float32)
    for b in range(B):
        out[b] = res.results[b]["h2T"].astype(np.float32).T
        fin[0, b, 0] = res.results[b]["h1tail"][:, -1].astype(np.float32)
        fin[1, b, 0] = out[b, -1]
    return out, fin
